# revision 1
# baseline (speedup 1.0000x reference)
"""Trainium2 Bass kernel v3 for nn_CrossGraphConvolution (hardware-loop design).

Backend model (measured on this setup): per-call cost is dominated by
per-STATIC-instruction overhead (~50-130us each, NEFF translation), while
dynamic execution runs at silicon speed. So the kernel is a ~150-static-
instruction body inside a For_i hardware loop over 8 m-windows of 512.

Math (per batch b, one NeuronCore each):
    S^T[n,m] = xn[:,n] . gn[:,m]        (cosine similarity, transposed)
    P^T = exp(S^T)                       (softmax numerator; max-subtract
                                          skipped: cosines are in [-1,1])
    o3[o,m] = sum_n xw[n,o] P^T[n,m]     (aggregation pre-projected by W,
                                          fp8 DoubleRow: 2 n-chunks/matmul)
    rows[m] = sum_n P^T[n,m]             (softmax denominator,
                                          gpsimd partition_all_reduce)
    y[o,m]  = LeakyReLU(o3)/rows * a + b (LeakyReLU commutes with the
                                          positive 1/rows scale; BN folded)

Host precomputes xn, gn (l2-normalized bf16), xw = (x^T W) in
[n-chunk-partition, o] layout (fp8e4), and BN a/b.
"""

import sys

import numpy as np

if "/opt/trn_rl_repo" not in sys.path:
    sys.path.insert(0, "/opt/trn_rl_repo")

B, C, N, M, OUT = 8, 128, 4096, 4096, 128
NJ = N // 128           # 32 n-chunks
MW = 1024               # m-window width (exp width; 2x512 matmul cols)
NMW = M // MW           # 8 m-windows
EPS_BN = 1e-5
NEG_SLOPE = 0.01


def _apply_bir_passes():
    """Ldweights dedup + single-wait legalization (same as baseline)."""
    import json

    import concourse.bass as bass

    if getattr(bass.Bass, "_bir_passes_applied", False):
        return
    orig = bass.Bass.to_json_bytes

    def patched(self):
        bir = json.loads(orig(self))
        for fn in bir.get("functions", []):
            for blk in fn.get("blocks", []):
                insts = blk.get("instructions", [])
                last_ldw = {}
                kept = []
                for ins in insts:
                    if ins.get("opcode") == "Ldweights":
                        eng = ins.get("engine")
                        key = json.dumps(
                            [
                                ins.get("ins"),
                                ins.get("perf_mode"),
                                ins.get("is_transpose"),
                                ins.get("tile_position"),
                            ],
                            sort_keys=True,
                        )
                        ow = (ins.get("sync_info") or {}).get("on_wait") or []
                        upd = (ins.get("sync_info") or {}).get("on_update") or []
                        if last_ldw.get(eng) == key and not upd:
                            if ow:
                                kept.append(
                                    {
                                        "debug": ins.get("debug", 0),
                                        "engine": eng,
                                        "ins": [],
                                        "name": ins["name"] + "-dedup",
                                        "opcode": "NoOp",
                                        "outs": [],
                                        "sync_info": {
                                            "on_update": [],
                                            "on_wait": ow,
                                        },
                                    }
                                )
                            continue
                        last_ldw[eng] = key
                    kept.append(ins)
                new_insts = []
                for ins in kept:
                    si = ins.get("sync_info")
                    ow = (si or {}).get("on_wait") or []
                    if len(ow) > 1:
                        for k, w in enumerate(ow[:-1]):
                            new_insts.append(
                                {
                                    "debug": ins.get("debug", 0),
                                    "engine": ins["engine"],
                                    "ins": [],
                                    "name": f"{ins['name']}-w{k}",
                                    "opcode": "NoOp",
                                    "outs": [],
                                    "sync_info": {
                                        "on_update": [],
                                        "on_wait": [w],
                                    },
                                }
                            )
                        si["on_wait"] = [ow[-1]]
                    new_insts.append(ins)
                blk["instructions"] = new_insts
        return json.dumps(bir).encode()

    bass.Bass.to_json_bytes = patched
    bass.Bass._bir_passes_applied = True


def _bcast(ap, parts):
    """Partition-stride-0 view of a [1, ...] DRAM AP, for DMA broadcast."""
    import concourse.bass as bass

    return bass.AP(
        tensor=ap.tensor,
        offset=ap.offset,
        ap=[[0, parts]] + [list(d) for d in ap.ap[1:]],
    )


def _ap3(sl, t_stride, t_n, m_stride, m_n):
    """3D AP view [partition][t][m] of a 2D tile slice (for DoubleRow)."""
    import concourse.bass as bass

    return bass.AP(
        tensor=sl.tensor,
        offset=sl.offset,
        ap=[list(sl.ap[0]), [t_stride, t_n], [m_stride, m_n]],
    )


def build_nc(repeats: int = 1, o3_mode: str = "dr", lrelu_mode: str = "dve",
             hints: bool = True):
    import concourse.bass as bass
    import concourse.tile as tile
    from concourse import bass_isa, mybir

    _apply_bir_passes()

    f32 = mybir.dt.float32
    bf16 = mybir.dt.bfloat16
    f8 = mybir.dt.float8e4
    ALU = mybir.AluOpType
    ACTF = mybir.ActivationFunctionType
    dr = o3_mode == "dr"
    xw_dt = f8 if dr else bf16
    pt_dt = f8 if dr else bf16

    nc = bass.Bass("TRN2")
    xn_d = nc.dram_tensor("xn", [C, N], bf16, kind="ExternalInput")
    gn_d = nc.dram_tensor("gn", [C, M], bf16, kind="ExternalInput")
    xw_d = nc.dram_tensor("xw", [128, NJ * OUT], xw_dt, kind="ExternalInput")
    ab_d = nc.dram_tensor("ab", [OUT, 2], f32, kind="ExternalInput")
    y_d = nc.dram_tensor("y", [OUT, M], bf16, kind="ExternalOutput")

    with tile.TileContext(nc) as tc:
        with (
            tc.tile_pool(name="const", bufs=1) as const,
            tc.tile_pool(name="sb", bufs=1) as sb,
            tc.tile_pool(name="rws", bufs=2) as rws,
            tc.tile_pool(name="ep", bufs=2) as ep,
            tc.tile_pool(name="drp", bufs=2, space="DRAM") as drp,
            tc.tile_pool(name="stp", bufs=2, space="PSUM") as stp,
            tc.tile_pool(name="o3p", bufs=1, space="PSUM") as o3p,
            tc.tile_pool(name="rwp", bufs=1, space="PSUM") as rwp,
        ):
            ab_sb = const.tile([OUT, 2], f32, tag="ab", name="ab_sb")
            nc.gpsimd.dma_start(out=ab_sb, in_=ab_d[:])
            ones_st = const.tile([128, 2], f8 if dr else bf16, tag="ones", name="ones_st")
            nc.vector.memset(ones_st, 1.0)
            ones_bc = const.tile([1, 128], f32, tag="onesbc", name="ones_bc")
            nc.vector.memset(ones_bc, 1.0)
            xn_sb = sb.tile([C, N], bf16, tag="xn", name="xn_sb")
            gn_sb = sb.tile([C, M], bf16, tag="gn", name="gn_sb")
            xw_sb = sb.tile([128, NJ * OUT], xw_dt, tag="xw", name="xw_sb")
            nc.gpsimd.dma_start(out=xn_sb, in_=xn_d[:])
            nc.gpsimd.dma_start(out=gn_sb, in_=gn_d[:])
            nc.gpsimd.dma_start(out=xw_sb, in_=xw_d[:])

            # output staging (dynamic DMA offsets don't compile; DVE writes
            # the window, one static DMA ships the full tensor at the end)
            y_all = sb.tile([OUT, M], bf16, tag="yall", name="y_all")
            # P^T for one m-window, n-chunk-major: pt[:, nj*MW + m]
            pt = sb.tile([128, NJ * MW], pt_dt, tag="pt", name="pt")

            def window_body(mw):
                o3 = o3p.tile([OUT, MW], f32, tag="o3", name="o3")
                rw_tiles = [
                    rwp.tile([1, 512], f32, tag=f"rw{h}", name=f"rowsum{h}")
                    for h in range(MW // 512)
                ]
                # stage the m-window of gn once: keeps the 32 S^T matmuls on
                # static APs (dynamic APs exhaust PE offset registers)
                gwin = ep.tile([C, MW], bf16, tag="gwin", name="gwin")
                nc.vector.tensor_copy(gwin, gn_sb[:, bass.ts(mw, MW)])
                for pj in range(NJ // 2):
                    for t in range(2):
                        nj = 2 * pj + t
                        st = stp.tile([128, MW], f32, tag="st", name="st")
                        for h in range(MW // 512):
                            nc.tensor.matmul(
                                st[:, h * 512 : (h + 1) * 512],
                                xn_sb[:, nj * 128 : (nj + 1) * 128],
                                gwin[:, h * 512 : (h + 1) * 512],
                                start=True,
                                stop=True,
                            )
                        nc.scalar.activation(
                            out=pt[:, nj * MW : (nj + 1) * MW],
                            in_=st,
                            func=ACTF.Exp,
                        )
                    if dr:
                        for h in range(MW // 512):
                            nc.tensor.matmul(
                                o3[:, h * 512 : (h + 1) * 512],
                                _ap3(
                                    xw_sb[:, pj * 256 : (pj + 1) * 256],
                                    128, 2, 1, 128,
                                ),
                                _ap3(pt[:, 2 * pj * MW + h * 512 :], MW, 2, 1, 512),
                                start=(pj == 0),
                                stop=(pj == NJ // 2 - 1),
                                perf_mode=mybir.MatmulPerfMode.DoubleRow,
                            )
                        # rows interleaved per pair: keeps the post-loop PE
                        # tail tiny so ACT never idles at the window boundary
                        for t in range(2):
                            nj = 2 * pj + t
                            for h in range(MW // 512):
                                nc.tensor.matmul(
                                    rw_tiles[h],
                                    ones_st[:, 0:1],
                                    pt[:, nj * MW + h * 512 : nj * MW + (h + 1) * 512],
                                    start=(nj == 0),
                                    stop=(nj == NJ - 1),
                                )
                    else:
                        for t in range(2):
                            nj = 2 * pj + t
                            for h in range(MW // 512):
                                nc.tensor.matmul(
                                    o3[:, h * 512 : (h + 1) * 512],
                                    xw_sb[:, nj * 128 : (nj + 1) * 128],
                                    pt[:, nj * MW + h * 512 : nj * MW + (h + 1) * 512],
                                    start=(nj == 0),
                                    stop=(nj == NJ - 1),
                                )

                # softmax denominator: ones-stationary matmuls over P^T
                # (consecutive, so the ones Ldweights dedups to one; DoubleRow
                # with a 1-row output miscompiles, so plain mode here)
                rr1 = rws.tile([1, MW], f32, tag="rr1", name="rr1")
                for h in range(MW // 512):
                    nc.vector.reciprocal(
                        out=rr1[:, h * 512 : (h + 1) * 512], in_=rw_tiles[h]
                    )
                # broadcast to 128 partitions via K=1 matmuls (DMA inside a
                # For_i body goes through the TriggerDma ISA path, which
                # miscompiles); output reuses a free st buffer, no extra PSUM
                rr = stp.tile([128, MW], f32, tag="st", name="rr")
                for h in range(MW // 512):
                    nc.tensor.matmul(
                        rr[:, h * 512 : (h + 1) * 512],
                        ones_bc[0:1, :],
                        rr1[:, h * 512 : (h + 1) * 512],
                        start=True,
                        stop=True,
                    )

                # epilogue: LeakyReLU, /rows, BN affine
                z = ep.tile([OUT, MW], f32, tag="z", name="z")
                if lrelu_mode == "act":
                    nc.scalar.activation(
                        out=z, in_=o3, func=ACTF.Lrelu, alpha=NEG_SLOPE
                    )
                else:
                    zt = ep.tile([OUT, MW], f32, tag="zt", name="zt")
                    nc.vector.tensor_scalar(
                        out=zt, in0=o3, scalar1=NEG_SLOPE, scalar2=None,
                        op0=ALU.mult,
                    )
                    nc.vector.tensor_tensor(out=z, in0=o3, in1=zt, op=ALU.max)
                del o3
                z2 = ep.tile([OUT, MW], f32, tag="z2", name="z2")
                nc.vector.tensor_tensor(out=z2, in0=z, in1=rr, op=ALU.mult)
                nc.vector.tensor_scalar(
                    out=y_all[:, bass.ts(mw, MW)],
                    in0=z2,
                    scalar1=ab_sb[:, 0:1],
                    scalar2=ab_sb[:, 1:2],
                    op0=ALU.mult,
                    op1=ALU.add,
                )

            hint_kw = (
                {"hint_engines": (mybir.EngineType.PE, mybir.EngineType.Activation)}
                if hints
                else {}
            )
            if repeats == 1:
                with tc.For_i(0, NMW, 1, **hint_kw) as mw:
                    window_body(mw)
            else:
                with tc.For_i(0, repeats, 1):
                    with tc.For_i(0, NMW, 1, **hint_kw) as mw:
                        window_body(mw)
            nc.gpsimd.dma_start(out=y_d[:], in_=y_all)
    return nc


_nc_cache: dict = {}


def _prep(input, target_g, weight, gamma, beta, running_mean, running_var):
    import ml_dtypes

    x = np.asarray(input, dtype=np.float32)
    g = np.asarray(target_g, dtype=np.float32)
    w = np.asarray(weight, dtype=np.float32)
    gamma = np.asarray(gamma, dtype=np.float32).reshape(OUT)
    beta = np.asarray(beta, dtype=np.float32).reshape(OUT)
    mean = np.asarray(running_mean, dtype=np.float32).reshape(OUT)
    var = np.asarray(running_var, dtype=np.float32).reshape(OUT)

    a_sc = (gamma / np.sqrt(var + EPS_BN)).astype(np.float32)
    b_sc = (beta - mean * a_sc).astype(np.float32)
    ab = np.ascontiguousarray(np.stack([a_sc, b_sc], axis=1))

    xn = x / np.maximum(np.sqrt((x * x).sum(axis=1, keepdims=True)), 1e-12)
    gn = g / np.maximum(np.sqrt((g * g).sum(axis=1, keepdims=True)), 1e-12)
    xn16 = np.ascontiguousarray(xn.astype(ml_dtypes.bfloat16))
    gn16 = np.ascontiguousarray(gn.astype(ml_dtypes.bfloat16))

    # xw[b, p, nj*128+o] = (x[b]^T @ W)[nj*128+p, o]
    xw = np.einsum("bcn,co->bno", x, w)
    xw = xw.reshape(B, NJ, 128, OUT).transpose(0, 2, 1, 3).reshape(B, 128, NJ * OUT)
    xw8 = np.ascontiguousarray(
        np.clip(xw, -224.0, 224.0).astype(ml_dtypes.float8_e4m3)
    )
    return [
        {"xn": xn16[b], "gn": gn16[b], "xw": xw8[b], "ab": ab} for b in range(B)
    ]


def kernel(input, target_g, weight, gamma, beta, running_mean, running_var):
    from concourse.bass_utils import run_bass_kernel_spmd

    if "nc" not in _nc_cache:
        _nc_cache["nc"] = build_nc(repeats=1)
    nc = _nc_cache["nc"]
    in_maps = _prep(
        input, target_g, weight, gamma, beta, running_mean, running_var
    )
    res = run_bass_kernel_spmd(nc, in_maps, core_ids=list(range(B)))
    return np.stack([res.results[b]["y"] for b in range(B)]).astype(np.float32)



# revision 20
# speedup vs baseline: 1.7612x; 1.7612x over previous
"""Trainium2 Bass kernel v4 for nn_CrossGraphConvolution (fully-unrolled,
3-engine-balanced design).

Math (per batch b, one NeuronCore each):
    S^T[n,m] = xn[:,n] . gn[:,m]        (cosine similarity, transposed)
    P^T = exp(S^T)                       (softmax numerator; max-subtract
                                          skipped: cosines are in [-1,1])
    o3[o,m] = sum_n xw[n,o] P^T[n,m]     (aggregation pre-projected by W,
                                          fp8 DoubleRow: 2 n-chunks/matmul)
    rows[m] = sum_n P^T[n,m]             (ones-stationary fp8 DoubleRow
                                          matmuls, 2-row output)
    y[o,m]  = LeakyReLU(o3)/rows * a + b (LeakyReLU commutes with the
                                          positive 1/rows scale; BN folded)

v4 structure (vs v3's window For_i):
  - 4 m-windows of 1024 fully unrolled: no per-window all-engine barrier
    (For_i inserts InstAllEngineBarrier per iteration), cross-window
    software pipelining, all static APs.
  - exp split across engines: most chunks on ACT (table exp -> fp8 store),
    the rest on DVE via Schraudolph-in-fp8-bit-space: one tensor_scalar
    uint8 = rne(s * 8*log2e + 56 - 0.344), bitcast-read as fp8e4m3.
    (Validated: final rel err 1.1e-3 vs 2e-2 budget.)
  - rows via DoubleRow ones-stationary matmuls (2-row output, halves the
    former plain rows cost); same moving AP as the o3 matmuls.
  - PE queue uses delayed-dependent-work order: the o3/rows matmuls of
    pair p are queued after the S^T matmuls of pair p+1, so PE never
    stalls waiting for exp.
  - Epilogue of window w (recip, lrelu, /rows, BN) is interleaved into
    window w+1's instruction streams.
"""

import sys

import numpy as np

if "/opt/trn_rl_repo" not in sys.path:
    sys.path.insert(0, "/opt/trn_rl_repo")

B, C, N, M, OUT = 8, 128, 4096, 4096, 128
NJ = N // 128           # 32 n-chunks
MW = 1024               # m-window width
NMW = M // MW           # 4 m-windows (fully unrolled)
PAIRS = NJ // 2         # 16 chunk-pairs per window
EPS_BN = 1e-5
NEG_SLOPE = 0.01

# Schraudolph exp in fp8e4m3 bit space: uint8 = rne(s*SCALE + BIAS),
# bitcast fp8e4m3.  (device store convert is RNE-saturating, measured)
SCH_SCALE = 8.0 * 1.4426950408889634
SCH_BIAS = 56.0 - 0.344

# chunks computed on DVE (Schraudolph); rest on ACT (table exp).
DVE_SET = frozenset({8, 10, 12, 14, 16, 18, 20, 22, 24, 26, 28, 29})


def _apply_bir_passes():
    """Ldweights dedup + single-wait legalization (same as v3)."""
    import json

    import concourse.bass as bass

    if getattr(bass.Bass, "_bir_passes_applied", False):
        return
    orig = bass.Bass.to_json_bytes

    def patched(self):
        bir = json.loads(orig(self))
        for fn in bir.get("functions", []):
            for blk in fn.get("blocks", []):
                insts = blk.get("instructions", [])
                last_ldw = {}
                kept = []
                for ins in insts:
                    if ins.get("opcode") == "Ldweights":
                        eng = ins.get("engine")
                        key = json.dumps(
                            [
                                ins.get("ins"),
                                ins.get("perf_mode"),
                                ins.get("is_transpose"),
                                ins.get("tile_position"),
                            ],
                            sort_keys=True,
                        )
                        ow = (ins.get("sync_info") or {}).get("on_wait") or []
                        upd = (ins.get("sync_info") or {}).get("on_update") or []
                        if last_ldw.get(eng) == key and not upd:
                            if ow:
                                kept.append(
                                    {
                                        "debug": ins.get("debug", 0),
                                        "engine": eng,
                                        "ins": [],
                                        "name": ins["name"] + "-dedup",
                                        "opcode": "NoOp",
                                        "outs": [],
                                        "sync_info": {
                                            "on_update": [],
                                            "on_wait": ow,
                                        },
                                    }
                                )
                            continue
                        last_ldw[eng] = key
                    kept.append(ins)
                new_insts = []
                for ins in kept:
                    si = ins.get("sync_info")
                    ow = (si or {}).get("on_wait") or []
                    if len(ow) > 1:
                        for k, w in enumerate(ow[:-1]):
                            new_insts.append(
                                {
                                    "debug": ins.get("debug", 0),
                                    "engine": ins["engine"],
                                    "ins": [],
                                    "name": f"{ins['name']}-w{k}",
                                    "opcode": "NoOp",
                                    "outs": [],
                                    "sync_info": {
                                        "on_update": [],
                                        "on_wait": [w],
                                    },
                                }
                            )
                        si["on_wait"] = [ow[-1]]
                    new_insts.append(ins)
                blk["instructions"] = new_insts
        return json.dumps(bir).encode()

    bass.Bass.to_json_bytes = patched
    bass.Bass._bir_passes_applied = True


def _ap3(sl, t_stride, t_n, m_stride, m_n):
    """3D AP view [partition][t][m] of a 2D tile slice (for DoubleRow)."""
    import concourse.bass as bass

    return bass.AP(
        tensor=sl.tensor,
        offset=sl.offset,
        ap=[list(sl.ap[0]), [t_stride, t_n], [m_stride, m_n]],
    )


def build_nc(repeats: int = 1, schraud: bool = True, rows_dr: bool = True):
    import concourse.bass as bass
    import concourse.tile as tile
    from concourse import mybir

    _apply_bir_passes()

    f32 = mybir.dt.float32
    bf16 = mybir.dt.bfloat16
    f8 = mybir.dt.float8e4
    u8 = mybir.dt.uint8
    ALU = mybir.AluOpType
    ACTF = mybir.ActivationFunctionType
    DR = mybir.MatmulPerfMode.DoubleRow

    dve_set = DVE_SET if schraud else frozenset()

    nc = bass.Bass("TRN2")
    xn_d = nc.dram_tensor("xn", [C, N], bf16, kind="ExternalInput")
    gn_d = nc.dram_tensor("gn", [C, M], bf16, kind="ExternalInput")
    xw_d = nc.dram_tensor("xw", [128, NJ * OUT], f8, kind="ExternalInput")
    ab_d = nc.dram_tensor("ab", [OUT, 2], f32, kind="ExternalInput")
    y_d = nc.dram_tensor("y", [OUT, M], bf16, kind="ExternalOutput")

    with tile.TileContext(nc) as tc:
        with (
            tc.tile_pool(name="const", bufs=1) as const,
            tc.tile_pool(name="sb", bufs=1) as sb,
            tc.tile_pool(name="ep", bufs=2) as ep,
            tc.tile_pool(name="stp", bufs=2, space="PSUM") as stp,
            tc.tile_pool(name="o3p", bufs=1, space="PSUM") as o3p,
            tc.tile_pool(name="rwp", bufs=1, space="PSUM") as rwp,
        ):
            ab_sb = const.tile([OUT, 2], f32, tag="ab", name="ab_sb")
            nc.gpsimd.dma_start(out=ab_sb, in_=ab_d[:])
            # all-ones DoubleRow stationary with FULL 128 columns: the rows
            # matmul then outputs the row-sum replicated on all 128
            # partitions -- the softmax-denominator broadcast comes free.
            ones_dr = const.tile([128, 256], f8, tag="onesdr", name="ones_dr")
            nc.vector.memset(ones_dr, 1.0)
            xn_sb = sb.tile([C, N], bf16, tag="xn", name="xn_sb")
            gn_sb = sb.tile([C, M], bf16, tag="gn", name="gn_sb")
            xw_sb = sb.tile([128, NJ * OUT], f8, tag="xw", name="xw_sb")
            nc.gpsimd.dma_start(out=xn_sb, in_=xn_d[:])
            nc.gpsimd.dma_start(out=gn_sb, in_=gn_d[:])
            nc.gpsimd.dma_start(out=xw_sb, in_=xw_d[:])

            y_all = sb.tile([OUT, M], bf16, tag="yall", name="y_all")
            # P^T staging, double-buffered by window parity
            pt_bufs = [
                sb.tile([128, NJ * MW], f8, tag=f"pt{i}", name=f"pt{i}")
                for i in range(2)
            ]

            def body():
                # per-window live state for the cross-window epilogue
                state = {}

                def emit_st(w, c):
                    """S^T matmuls for chunk c of window w -> st psum tile."""
                    st = stp.tile([128, MW], f32, tag="st", name="st")
                    for h in range(2):
                        nc.tensor.matmul(
                            st[:, h * 512 : (h + 1) * 512],
                            xn_sb[:, c * 128 : (c + 1) * 128],
                            gn_sb[:, w * MW + h * 512 : w * MW + (h + 1) * 512],
                            start=True,
                            stop=True,
                        )
                    return st

                def emit_consumer(w, c, st):
                    pt = pt_bufs[w % 2]
                    out_sl = pt[:, c * MW : (c + 1) * MW]
                    if c in dve_set:
                        nc.vector.tensor_scalar(
                            out=out_sl.bitcast(u8),
                            in0=st,
                            scalar1=SCH_SCALE,
                            scalar2=SCH_BIAS,
                            op0=ALU.mult,
                            op1=ALU.add,
                        )
                    else:
                        nc.scalar.activation(out=out_sl, in_=st, func=ACTF.Exp)

                def emit_dep(w, pj, o3, rw):
                    """o3 + rows DoubleRow matmuls for pair pj of window w."""
                    pt = pt_bufs[w % 2]
                    start = pj == 0
                    stop = pj == PAIRS - 1
                    for h in range(2):
                        nc.tensor.matmul(
                            o3[:, h * 512 : (h + 1) * 512],
                            _ap3(xw_sb[:, pj * 256 : (pj + 1) * 256], 128, 2, 1, 128),
                            _ap3(pt[:, 2 * pj * MW + h * 512 :], MW, 2, 1, 512),
                            start=start,
                            stop=stop,
                            perf_mode=DR,
                        )
                    if rows_dr:
                        for h in range(2):
                            nc.tensor.matmul(
                                rw[h],
                                _ap3(ones_dr[:, 0:256], 128, 2, 1, 128),
                                _ap3(pt[:, 2 * pj * MW + h * 512 :], MW, 2, 1, 512),
                                start=start,
                                stop=stop,
                                perf_mode=DR,
                            )
                    else:
                        for t in range(2):
                            c = 2 * pj + t
                            for h in range(2):
                                nc.tensor.matmul(
                                    rw[h],
                                    ones_dr[:, 0:128],
                                    pt[:, c * MW + h * 512 : c * MW + (h + 1) * 512],
                                    start=(c == 0),
                                    stop=(c == NJ - 1),
                                )

                # epilogue pieces for window w, injected into window w+1's
                # streams (or emitted serially for the last window)
                def ep_recip(w):
                    s = state[w]
                    s["rr"] = ep.tile([128, MW], bf16, tag="rr", name="rr")
                    with nc.allow_low_precision(
                        reason="1/rowsum in bf16: 0.4% rel err, budget 2e-2"
                    ):
                        for h in range(2):
                            nc.vector.reciprocal(
                                out=s["rr"][:, h * 512 : (h + 1) * 512],
                                in_=s["rw"][h],
                            )

                def ep_lrelu(w):
                    s = state[w]
                    o3 = s["o3"]
                    s["zt"] = ep.tile([OUT, MW], f32, tag="zt", name="zt")
                    nc.vector.tensor_scalar(
                        out=s["zt"], in0=o3, scalar1=NEG_SLOPE, scalar2=None,
                        op0=ALU.mult,
                    )
                    s["z"] = ep.tile([OUT, MW], f32, tag="z", name="z")
                    nc.vector.tensor_tensor(
                        out=s["z"], in0=o3, in1=s["zt"], op=ALU.max
                    )

                def ep_z2(w):
                    s = state[w]
                    s["z2"] = ep.tile([OUT, MW], f32, tag="z2", name="z2")
                    nc.vector.tensor_tensor(
                        out=s["z2"], in0=s["z"], in1=s["rr"], op=ALU.mult
                    )

                def ep_y(w):
                    s = state[w]
                    nc.vector.tensor_scalar(
                        out=y_all[:, w * MW : (w + 1) * MW],
                        in0=s["z2"],
                        scalar1=ab_sb[:, 0:1],
                        scalar2=ab_sb[:, 1:2],
                        op0=ALU.mult,
                        op1=ALU.add,
                    )

                # injection map: slot -> list of (engine-op emitters)
                def injections(w, slot):
                    if w < 1:
                        return
                    pw = w - 1
                    if slot == 0:
                        ep_recip(pw)
                    elif slot == 1:
                        ep_lrelu(pw)
                    elif slot == 2:
                        ep_z2(pw)
                    elif slot == 3:
                        ep_y(pw)

                for w in range(NMW):
                    o3 = o3p.tile([OUT, MW], f32, tag="o3", name="o3")
                    rw = [
                        rwp.tile([128, 512], f32, tag=f"rw{h}", name=f"rw{h}")
                        for h in range(2)
                    ]
                    state[w] = {"o3": o3, "rw": rw}
                    for s in range(PAIRS):
                        c0, c1 = 2 * s, 2 * s + 1
                        st0 = emit_st(w, c0)
                        st1 = emit_st(w, c1)
                        injections(w, s)
                        if s >= 1:
                            emit_dep(w, s - 1, o3, rw)
                        emit_consumer(w, c0, st0)
                        emit_consumer(w, c1, st1)
                    emit_dep(w, PAIRS - 1, o3, rw)
                    if w >= 1:
                        del state[w - 1]

                # final window's epilogue (serial tail)
                wl = NMW - 1
                ep_recip(wl)
                ep_lrelu(wl)
                ep_z2(wl)
                ep_y(wl)

            if repeats == 1:
                body()
            else:
                with tc.For_i(0, repeats, 1):
                    body()
            nc.gpsimd.dma_start(out=y_d[:], in_=y_all)
    return nc


_nc_cache: dict = {}


def _prep(input, target_g, weight, gamma, beta, running_mean, running_var):
    import ml_dtypes

    x = np.asarray(input, dtype=np.float32)
    g = np.asarray(target_g, dtype=np.float32)
    w = np.asarray(weight, dtype=np.float32)
    gamma = np.asarray(gamma, dtype=np.float32).reshape(OUT)
    beta = np.asarray(beta, dtype=np.float32).reshape(OUT)
    mean = np.asarray(running_mean, dtype=np.float32).reshape(OUT)
    var = np.asarray(running_var, dtype=np.float32).reshape(OUT)

    a_sc = (gamma / np.sqrt(var + EPS_BN)).astype(np.float32)
    b_sc = (beta - mean * a_sc).astype(np.float32)
    ab = np.ascontiguousarray(np.stack([a_sc, b_sc], axis=1))

    xn = x / np.maximum(np.sqrt((x * x).sum(axis=1, keepdims=True)), 1e-12)
    gn = g / np.maximum(np.sqrt((g * g).sum(axis=1, keepdims=True)), 1e-12)
    xn16 = np.ascontiguousarray(xn.astype(ml_dtypes.bfloat16))
    gn16 = np.ascontiguousarray(gn.astype(ml_dtypes.bfloat16))

    # xw[b, p, nj*128+o] = (x[b]^T @ W)[nj*128+p, o]
    xw = np.einsum("bcn,co->bno", x, w)
    xw = xw.reshape(B, NJ, 128, OUT).transpose(0, 2, 1, 3).reshape(B, 128, NJ * OUT)
    xw8 = np.ascontiguousarray(
        np.clip(xw, -224.0, 224.0).astype(ml_dtypes.float8_e4m3)
    )
    return [
        {"xn": xn16[b], "gn": gn16[b], "xw": xw8[b], "ab": ab} for b in range(B)
    ]


def kernel(input, target_g, weight, gamma, beta, running_mean, running_var):
    from concourse.bass_utils import run_bass_kernel_spmd

    if "nc" not in _nc_cache:
        _nc_cache["nc"] = build_nc(repeats=1)
    nc = _nc_cache["nc"]
    in_maps = _prep(
        input, target_g, weight, gamma, beta, running_mean, running_var
    )
    res = run_bass_kernel_spmd(nc, in_maps, core_ids=list(range(B)))
    return np.stack([res.results[b]["y"] for b in range(B)]).astype(np.float32)


# revision 24
# speedup vs baseline: 1.9206x; 1.0905x over previous
"""Trainium2 Bass kernel v4 for nn_CrossGraphConvolution (fully-unrolled,
3-engine-balanced design).

Math (per batch b, one NeuronCore each):
    S^T[n,m] = xn[:,n] . gn[:,m]        (cosine similarity, transposed)
    P^T = exp(S^T)                       (softmax numerator; max-subtract
                                          skipped: cosines are in [-1,1])
    o3[o,m] = sum_n xw[n,o] P^T[n,m]     (aggregation pre-projected by W,
                                          fp8 DoubleRow: 2 n-chunks/matmul)
    rows[m] = sum_n P^T[n,m]             (ones-stationary fp8 DoubleRow
                                          matmuls, 2-row output)
    y[o,m]  = LeakyReLU(o3)/rows * a + b (LeakyReLU commutes with the
                                          positive 1/rows scale; BN folded)

v4 structure (vs v3's window For_i):
  - 4 m-windows of 1024 fully unrolled: no per-window all-engine barrier
    (For_i inserts InstAllEngineBarrier per iteration), cross-window
    software pipelining, all static APs.
  - exp split across engines: most chunks on ACT (table exp -> fp8 store),
    the rest on DVE via Schraudolph-in-fp8-bit-space: one tensor_scalar
    uint8 = rne(s * 8*log2e + 56 - 0.344), bitcast-read as fp8e4m3.
    (Validated: final rel err 1.1e-3 vs 2e-2 budget.)
  - rows via DoubleRow ones-stationary matmuls (2-row output, halves the
    former plain rows cost); same moving AP as the o3 matmuls.
  - PE queue uses delayed-dependent-work order: the o3/rows matmuls of
    pair p are queued after the S^T matmuls of pair p+1, so PE never
    stalls waiting for exp.
  - Epilogue of window w (recip, lrelu, /rows, BN) is interleaved into
    window w+1's instruction streams.
"""

import sys

import numpy as np

if "/opt/trn_rl_repo" not in sys.path:
    sys.path.insert(0, "/opt/trn_rl_repo")

B, C, N, M, OUT = 8, 128, 4096, 4096, 128
NJ = N // 128           # 32 n-chunks
MW = 1024               # m-window width
NMW = M // MW           # 4 m-windows (fully unrolled)
PAIRS = NJ // 2         # 16 chunk-pairs per window
EPS_BN = 1e-5
NEG_SLOPE = 0.01

# Schraudolph exp in fp8e4m3 bit space: uint8 = rne(s*SCALE + BIAS),
# bitcast fp8e4m3.  (device store convert is RNE-saturating, measured)
SCH_SCALE = 8.0 * 1.4426950408889634
SCH_BIAS = 56.0 - 0.344

# chunks computed on DVE (Schraudolph); rest on ACT (table exp).
# avoids slots where DVE carries epilogue injections (0: lrelu, 8-10:
# recip/z2/y) and the last pair (tail-critical).
DVE_SET = frozenset({3, 5, 7, 9, 11, 13, 15, 22, 24, 26, 28, 29})


def _apply_bir_passes():
    """Ldweights dedup + single-wait legalization (same as v3)."""
    import json

    import concourse.bass as bass

    if getattr(bass.Bass, "_bir_passes_applied", False):
        return
    orig = bass.Bass.to_json_bytes

    def patched(self):
        bir = json.loads(orig(self))
        for fn in bir.get("functions", []):
            for blk in fn.get("blocks", []):
                insts = blk.get("instructions", [])
                last_ldw = {}
                kept = []
                for ins in insts:
                    if ins.get("opcode") == "Ldweights":
                        eng = ins.get("engine")
                        key = json.dumps(
                            [
                                ins.get("ins"),
                                ins.get("perf_mode"),
                                ins.get("is_transpose"),
                                ins.get("tile_position"),
                            ],
                            sort_keys=True,
                        )
                        ow = (ins.get("sync_info") or {}).get("on_wait") or []
                        upd = (ins.get("sync_info") or {}).get("on_update") or []
                        if last_ldw.get(eng) == key and not upd:
                            if ow:
                                kept.append(
                                    {
                                        "debug": ins.get("debug", 0),
                                        "engine": eng,
                                        "ins": [],
                                        "name": ins["name"] + "-dedup",
                                        "opcode": "NoOp",
                                        "outs": [],
                                        "sync_info": {
                                            "on_update": [],
                                            "on_wait": ow,
                                        },
                                    }
                                )
                            continue
                        last_ldw[eng] = key
                    kept.append(ins)
                new_insts = []
                for ins in kept:
                    si = ins.get("sync_info")
                    ow = (si or {}).get("on_wait") or []
                    if len(ow) > 1:
                        for k, w in enumerate(ow[:-1]):
                            new_insts.append(
                                {
                                    "debug": ins.get("debug", 0),
                                    "engine": ins["engine"],
                                    "ins": [],
                                    "name": f"{ins['name']}-w{k}",
                                    "opcode": "NoOp",
                                    "outs": [],
                                    "sync_info": {
                                        "on_update": [],
                                        "on_wait": [w],
                                    },
                                }
                            )
                        si["on_wait"] = [ow[-1]]
                    new_insts.append(ins)
                blk["instructions"] = new_insts
        return json.dumps(bir).encode()

    bass.Bass.to_json_bytes = patched
    bass.Bass._bir_passes_applied = True


def _ap3(sl, t_stride, t_n, m_stride, m_n):
    """3D AP view [partition][t][m] of a 2D tile slice (for DoubleRow)."""
    import concourse.bass as bass

    return bass.AP(
        tensor=sl.tensor,
        offset=sl.offset,
        ap=[list(sl.ap[0]), [t_stride, t_n], [m_stride, m_n]],
    )


def build_nc(repeats: int = 1, schraud: bool = True, rows_dr: bool = True):
    import concourse.bass as bass
    import concourse.tile as tile
    from concourse import mybir

    _apply_bir_passes()

    f32 = mybir.dt.float32
    bf16 = mybir.dt.bfloat16
    f8 = mybir.dt.float8e4
    u8 = mybir.dt.uint8
    ALU = mybir.AluOpType
    ACTF = mybir.ActivationFunctionType
    DR = mybir.MatmulPerfMode.DoubleRow

    dve_set = DVE_SET if schraud else frozenset()

    nc = bass.Bass("TRN2")
    xn_d = nc.dram_tensor("xn", [C, N], bf16, kind="ExternalInput")
    gn_d = nc.dram_tensor("gn", [C, M], bf16, kind="ExternalInput")
    xw_d = nc.dram_tensor("xw", [128, NJ * OUT], f8, kind="ExternalInput")
    ab_d = nc.dram_tensor("ab", [OUT, 2], f32, kind="ExternalInput")
    y_d = nc.dram_tensor("y", [OUT, M], bf16, kind="ExternalOutput")

    with tile.TileContext(nc) as tc:
        with (
            tc.tile_pool(name="const", bufs=1) as const,
            tc.tile_pool(name="sb", bufs=1) as sb,
            tc.tile_pool(name="ep", bufs=2) as ep,
            tc.tile_pool(name="stp", bufs=2, space="PSUM") as stp,
            tc.tile_pool(name="o3p", bufs=1, space="PSUM") as o3p,
            tc.tile_pool(name="rwp", bufs=1, space="PSUM") as rwp,
        ):
            ab_sb = const.tile([OUT, 2], f32, tag="ab", name="ab_sb")
            nc.gpsimd.dma_start(out=ab_sb, in_=ab_d[:])
            # all-ones DoubleRow stationary with FULL 128 columns: the rows
            # matmul then outputs the row-sum replicated on all 128
            # partitions -- the softmax-denominator broadcast comes free.
            ones_dr = const.tile([128, 256], f8, tag="onesdr", name="ones_dr")
            nc.vector.memset(ones_dr, 1.0)
            xn_sb = sb.tile([C, N], bf16, tag="xn", name="xn_sb")
            gn_sb = sb.tile([C, M], bf16, tag="gn", name="gn_sb")
            xw_sb = sb.tile([128, NJ * OUT], f8, tag="xw", name="xw_sb")
            nc.gpsimd.dma_start(out=xn_sb, in_=xn_d[:])
            nc.gpsimd.dma_start(out=gn_sb, in_=gn_d[:])
            nc.gpsimd.dma_start(out=xw_sb, in_=xw_d[:])

            y_all = sb.tile([OUT, M], bf16, tag="yall", name="y_all")
            # P^T staging, double-buffered by window parity
            pt_bufs = [
                sb.tile([128, NJ * MW], f8, tag=f"pt{i}", name=f"pt{i}")
                for i in range(2)
            ]

            def body():
                # per-window live state for the cross-window epilogue
                state = {}

                def emit_st(w, c):
                    """S^T matmuls for chunk c of window w -> st psum tile."""
                    st = stp.tile([128, MW], f32, tag="st", name="st")
                    for h in range(2):
                        nc.tensor.matmul(
                            st[:, h * 512 : (h + 1) * 512],
                            xn_sb[:, c * 128 : (c + 1) * 128],
                            gn_sb[:, w * MW + h * 512 : w * MW + (h + 1) * 512],
                            start=True,
                            stop=True,
                        )
                    return st

                def emit_consumer(w, c, st):
                    pt = pt_bufs[w % 2]
                    out_sl = pt[:, c * MW : (c + 1) * MW]
                    if c in dve_set:
                        nc.vector.tensor_scalar(
                            out=out_sl.bitcast(u8),
                            in0=st,
                            scalar1=SCH_SCALE,
                            scalar2=SCH_BIAS,
                            op0=ALU.mult,
                            op1=ALU.add,
                        )
                    else:
                        nc.scalar.activation(out=out_sl, in_=st, func=ACTF.Exp)

                def emit_o3(w, pj):
                    """o3 DoubleRow matmuls for pair pj of window w."""
                    pt = pt_bufs[w % 2]
                    o3 = state[w]["o3"]
                    for h in range(2):
                        nc.tensor.matmul(
                            o3[:, h * 512 : (h + 1) * 512],
                            _ap3(xw_sb[:, pj * 256 : (pj + 1) * 256], 128, 2, 1, 128),
                            _ap3(pt[:, 2 * pj * MW + h * 512 :], MW, 2, 1, 512),
                            start=pj == 0,
                            stop=pj == PAIRS - 1,
                            perf_mode=DR,
                        )

                def emit_rows(w, pairs):
                    """rows matmuls for a block of pairs of window w.

                    h-major so all MMs in the block share one deduped
                    all-ones LDWEIGHTS; accumulates rowsum broadcast to all
                    128 partitions."""
                    if rows_dr == "off":
                        return
                    pt = pt_bufs[w % 2]
                    rw = state[w]["rw"]
                    for pj in pairs:
                        for h in range(2):
                            nc.tensor.matmul(
                                rw[h],
                                _ap3(ones_dr[:, 0:256], 128, 2, 1, 128),
                                _ap3(pt[:, 2 * pj * MW + h * 512 :], MW, 2, 1, 512),
                                start=pj == 0,
                                stop=pj == PAIRS - 1,
                                perf_mode=DR,
                            )

                # epilogue pieces for window w, injected into window w+1's
                # streams (or emitted serially for the last window)
                def ep_recip(w):
                    s = state[w]
                    s["rr"] = ep.tile([128, MW], bf16, tag="rr", name="rr")
                    with nc.allow_low_precision(
                        reason="1/rowsum in bf16: 0.4% rel err, budget 2e-2"
                    ):
                        for h in range(2):
                            nc.vector.reciprocal(
                                out=s["rr"][:, h * 512 : (h + 1) * 512],
                                in_=s["rw"][h],
                            )

                def ep_lrelu(w):
                    s = state[w]
                    o3 = s["o3"]
                    s["zt"] = ep.tile([OUT, MW], f32, tag="zt", name="zt")
                    nc.vector.tensor_scalar(
                        out=s["zt"], in0=o3, scalar1=NEG_SLOPE, scalar2=None,
                        op0=ALU.mult,
                    )
                    s["z"] = ep.tile([OUT, MW], f32, tag="z", name="z")
                    nc.vector.tensor_tensor(
                        out=s["z"], in0=o3, in1=s["zt"], op=ALU.max
                    )

                def ep_z2(w):
                    s = state[w]
                    s["z2"] = ep.tile([OUT, MW], f32, tag="z2", name="z2")
                    nc.vector.tensor_tensor(
                        out=s["z2"], in0=s["z"], in1=s["rr"], op=ALU.mult
                    )

                def ep_y(w):
                    s = state[w]
                    nc.vector.tensor_scalar(
                        out=y_all[:, w * MW : (w + 1) * MW],
                        in0=s["z2"],
                        scalar1=ab_sb[:, 0:1],
                        scalar2=ab_sb[:, 1:2],
                        op0=ALU.mult,
                        op1=ALU.add,
                    )

                # injection map: epilogue(w-1) pieces into window w's streams
                def injections(w, slot):
                    if w < 1:
                        return
                    pw = w - 1
                    if slot == 0:
                        ep_lrelu(pw)
                    elif slot == 8:
                        ep_recip(pw)
                    elif slot == 9:
                        ep_z2(pw)
                    elif slot == 10:
                        ep_y(pw)

                # rows-block schedule: (win, slot) -> [(src_w, pairs)]
                # window w's rows run inside window w+1 (pt persists), in
                # blocks of 4 pairs so the ones-LDW dedups per block; the
                # last window self-schedules late + tail.
                wl = NMW - 1
                rows_sched: dict = {}
                blocks = [list(range(4 * k, 4 * k + 4)) for k in range(4)]
                for w in range(NMW):
                    if w < wl:
                        for k in range(4):
                            rows_sched.setdefault((w + 1, 1 + 2 * k), []).append(
                                (w, blocks[k])
                            )
                    else:
                        for k in range(3):
                            rows_sched.setdefault((w, 9 + 2 * k), []).append(
                                (w, blocks[k])
                            )

                for w in range(NMW):
                    state[w] = {
                        "o3": o3p.tile([OUT, MW], f32, tag="o3", name="o3"),
                        "rw": [
                            rwp.tile([128, 512], f32, tag=f"rw{h}", name=f"rw{h}")
                            for h in range(2)
                        ],
                    }
                    for s in range(PAIRS):
                        c0, c1 = 2 * s, 2 * s + 1
                        st0 = emit_st(w, c0)
                        st1 = emit_st(w, c1)
                        if s >= 2:
                            emit_o3(w, s - 2)
                        for src_w, prs in rows_sched.get((w, s), []):
                            emit_rows(src_w, prs)
                        injections(w, s)
                        emit_consumer(w, c0, st0)
                        emit_consumer(w, c1, st1)
                    emit_o3(w, PAIRS - 2)
                    emit_o3(w, PAIRS - 1)
                    if w >= 2:
                        del state[w - 2]

                # final window's tail: last rows block + epilogue (serial)
                emit_rows(wl, blocks[3])
                ep_recip(wl)
                ep_lrelu(wl)
                ep_z2(wl)
                ep_y(wl)

            if repeats == 1:
                body()
            else:
                with tc.For_i(0, repeats, 1):
                    body()
            nc.gpsimd.dma_start(out=y_d[:], in_=y_all)
    return nc


_nc_cache: dict = {}


def _prep(input, target_g, weight, gamma, beta, running_mean, running_var):
    import ml_dtypes

    x = np.asarray(input, dtype=np.float32)
    g = np.asarray(target_g, dtype=np.float32)
    w = np.asarray(weight, dtype=np.float32)
    gamma = np.asarray(gamma, dtype=np.float32).reshape(OUT)
    beta = np.asarray(beta, dtype=np.float32).reshape(OUT)
    mean = np.asarray(running_mean, dtype=np.float32).reshape(OUT)
    var = np.asarray(running_var, dtype=np.float32).reshape(OUT)

    a_sc = (gamma / np.sqrt(var + EPS_BN)).astype(np.float32)
    b_sc = (beta - mean * a_sc).astype(np.float32)
    ab = np.ascontiguousarray(np.stack([a_sc, b_sc], axis=1))

    xn = x / np.maximum(np.sqrt((x * x).sum(axis=1, keepdims=True)), 1e-12)
    gn = g / np.maximum(np.sqrt((g * g).sum(axis=1, keepdims=True)), 1e-12)
    xn16 = np.ascontiguousarray(xn.astype(ml_dtypes.bfloat16))
    gn16 = np.ascontiguousarray(gn.astype(ml_dtypes.bfloat16))

    # xw[b, p, nj*128+o] = (x[b]^T @ W)[nj*128+p, o]
    xw = np.einsum("bcn,co->bno", x, w)
    xw = xw.reshape(B, NJ, 128, OUT).transpose(0, 2, 1, 3).reshape(B, 128, NJ * OUT)
    xw8 = np.ascontiguousarray(
        np.clip(xw, -224.0, 224.0).astype(ml_dtypes.float8_e4m3)
    )
    return [
        {"xn": xn16[b], "gn": gn16[b], "xw": xw8[b], "ab": ab} for b in range(B)
    ]


def kernel(input, target_g, weight, gamma, beta, running_mean, running_var):
    from concourse.bass_utils import run_bass_kernel_spmd

    if "nc" not in _nc_cache:
        _nc_cache["nc"] = build_nc(repeats=1)
    nc = _nc_cache["nc"]
    in_maps = _prep(
        input, target_g, weight, gamma, beta, running_mean, running_var
    )
    res = run_bass_kernel_spmd(nc, in_maps, core_ids=list(range(B)))
    return np.stack([res.results[b]["y"] for b in range(B)]).astype(np.float32)
